# revision 26
# baseline (speedup 1.0000x reference)
"""Trainium2 Bass kernel for 2-layer AGRU (nn_AGRU_48584670052560).

The recurrence is strongly chaotic (a 1e-7 relative input perturbation
decorrelates the output by t=511), so the kernel must be faithful-fp32:
every matmul, activation and elementwise op runs in fp32; any bf16
shortcut saturates the error far above the fp32 envelope.

Structure (per core, data-parallel over batch, B=64 -> 8 x BL=8):
  P0: proj0 = x @ [Wz0;Wr0;Vh0].T          (batched rows, fp32)
  S0: 512-step scan, layer 0               (fp32)
  P1: proj1 = y0 @ [Wz1;Wr1;Vh1].T         (batched rows, fp32)
  S1: 512-step scan, layer 1 -> output

Scan-step dataflow: h.T is the stationary matmul operand (fp32
self-loading matmul); W.T slices stream. The four 512-wide N-chunks of
the z|r preactivation go to four PSUM partition strips (32j..32j+8) of
one PSUM bank via tile_position col packing so the streams overlap in
the PE array. Preactivations + projections are summed by DVE during
PSUM evacuation; gates come back to the H-on-partitions layout via
PE-mode transposes (fp32); all elementwise runs 128-partitions-wide.
Phases are sequential because both layers' fp32 U-matrices cannot be
SBUF-resident at once; one 96KB/partition weight slot is reused by all
four phases.
"""

import numpy as np

S, B, I, H, L = 512, 64, 512, 1024, 2
NCORES = 8
BL = B // NCORES  # 8
GAMMA = 0.01
UNROLL = 8  # steps per For_i body


def _build(S_=S, eps=(1.0, 1.0), with_bias=(False, False)):
    import concourse.mybir as mybir
    import concourse.tile as tile
    import concourse.bass as bass
    from concourse import bacc
    from contextlib import ExitStack

    f32 = mybir.dt.float32
    AF = mybir.ActivationFunctionType

    NT = S_
    R = S_ * BL  # rows = (t, b), t-major
    assert NT % UNROLL == 0

    nc = bacc.Bacc(None, target_bir_lowering=False)

    # ---- dram parameters (all fp32) ----
    xT_p = nc.declare_dram_parameter("xT", [I // 128, 128, R], f32, isOutput=False)
    wall_p = [
        nc.declare_dram_parameter("wall0", [I // 128, 128, 3 * H], f32, isOutput=False),
        nc.declare_dram_parameter("wall1", [H // 128, 128, 3 * H], f32, isOutput=False),
    ]
    uzr_p = [
        nc.declare_dram_parameter(f"uzr{l}", [8, 128, 2 * H], f32, isOutput=False)
        for l in range(2)
    ]
    aT_p = [
        nc.declare_dram_parameter(f"aT{l}", [8, 128, H], f32, isOutput=False)
        for l in range(2)
    ]
    h0T_p = nc.declare_dram_parameter("h0T", [2, 8, 128, BL], f32, isOutput=False)
    ident_p = nc.declare_dram_parameter("ident", [128, 128], f32, isOutput=False)
    biasT_p = []
    for l in range(2):
        if with_bias[l]:
            biasT_p.append(
                nc.declare_dram_parameter(f"biasT{l}", [2, 128, 512], f32, isOutput=False)
            )
        else:
            biasT_p.append(None)
    y1T_o = nc.declare_dram_parameter("y1T", [NT, 8, 128, BL], f32, isOutput=True)
    hnT_o = nc.declare_dram_parameter("hnT", [2, 8, 128, BL], f32, isOutput=True)

    with ExitStack() as ctx:
        tc = ctx.enter_context(tile.TileContext(nc))
        wpool = ctx.enter_context(tc.tile_pool(name="weights", bufs=1))
        state = ctx.enter_context(tc.tile_pool(name="state", bufs=1))
        dram = ctx.enter_context(tc.tile_pool(name="dram", bufs=1, space="DRAM"))
        spsum = ctx.enter_context(tc.tile_pool(name="spsum", bufs=1, space="PSUM"))
        ppsum = ctx.enter_context(tc.tile_pool(name="ppsum", bufs=2, space="PSUM"))
        stage = ctx.enter_context(tc.tile_pool(name="stage", bufs=2))
        xload = ctx.enter_context(tc.tile_pool(name="xload", bufs=3))

        # ---- DRAM scratch ----
        proj = [dram.tile([R, 3 * H], f32, tag=f"proj{l}", name=f"proj{l}") for l in range(2)]
        y0T = dram.tile([8, 128, NT, BL], f32, tag="y0T")

        # ---- persistent small tiles ----
        ident = state.tile([128, 128], f32, tag="ident", name="ident")
        nc.sync.dma_start(ident[:], ident_p[:])
        ident8 = state.tile([8, 8], f32, tag="ident8", name="ident8")
        nc.sync.dma_start(ident8[:], ident_p[0:8, 0:8])
        hT = [state.tile([128, 8, BL], f32, tag=f"hT{l}", name=f"hT{l}") for l in range(2)]
        for l in range(2):
            nc.sync.dma_start(hT[l][:], h0T_p[l].rearrange("hh p b -> p hh b"))
        biasT_sb = []
        for l in range(2):
            if with_bias[l]:
                bt = state.tile([2, 128, 512], f32, tag=f"biasT{l}", name=f"biasTs{l}")
                nc.sync.dma_start(bt[:], biasT_p[l][:])
                biasT_sb.append(bt)
            else:
                biasT_sb.append(None)

        # persistent scan psum/addend tiles (memset once; strips rewritten)
        pzr_t = spsum.tile([128, 512], f32, tag="pzr", name="pzr")
        pdh_t = spsum.tile([128, 512], f32, tag="pdh", name="pdh")
        nc.vector.memset(pzr_t[:], 0.0)
        nc.vector.memset(pdh_t[:], 0.0)
        sxrow_t = [state.tile([8, 3 * H], f32, tag=f"sxrow{a}", name=f"sxrow{a}") for a in range(2)]

        # ---- one 96KB/partition weight slot shared by all phases ----
        def wslot():
            return wpool.tile([128, 8, 3 * H], f32, tag="bigw", name="bigw")

        # ---- projection phase emitter ----
        def emit_proj(layer, wsrc_k, lhs_loader, out_dram):
            w = wslot()
            for k in range(wsrc_k):
                nc.sync.dma_start(w[:, k, :], wall_p[layer][k])
            nm = R // 128
            for m in range(nm):
                lt = lhs_loader(m)
                for n in range(6):
                    ps = ppsum.tile([128, 512], f32, tag="pj")
                    for k in range(wsrc_k):
                        nc.tensor.matmul(
                            ps[:],
                            lt[:, k, :],
                            w[:, k, bass.ts(n, 512)],
                            start=(k == 0),
                            stop=(k == wsrc_k - 1),
                        )
                    ob = stage.tile([128, 512], f32, tag="pjout", bufs=3)
                    if n % 2 == 0:
                        nc.scalar.activation(ob[:], ps[:], AF.Copy)
                    else:
                        nc.vector.tensor_copy(ob[:], ps[:])
                    nc.sync.dma_start(
                        out_dram[bass.ts(m, 128), bass.ts(n, 512)], ob[:]
                    )

        # P0
        def p0_lhs(m):
            xt = xload.tile([128, I // 128, 128], f32, tag="xt")
            for k in range(I // 128):
                nc.sync.dma_start(xt[:, k, :], xT_p[k][:, bass.ts(m, 128)])
            return xt

        emit_proj(0, I // 128, p0_lhs, proj[0])

        # ---- scan phase emitter ----
        def emit_scan(l, w):
            # w: weight slot with uzr in [:, :, 0:2048], aT in [:, :, 2048:3072]
            def emit_step(t, par):
                # one contiguous prefetch of this step's projections (rows layout)
                sxrow = sxrow_t[par]
                nc.sync.dma_start(sxrow[:], proj[l][bass.ts(t, BL), :])
                # z|r preactivation: ident-MM seeds PSUM with the projection
                # (PE work, independent of h -> fills PE gaps), then 4 strip
                # chains accumulate h @ Uzr.T, k-interleaved
                for j in range(4):
                    nc.tensor.matmul(
                        pzr_t[32 * j : 32 * j + BL, :],
                        ident8[:],
                        sxrow[:, bass.ts(j, 512)],
                        start=True,
                        stop=False,
                        tile_position=(0, 32 * j),
                        skip_group_check=True,
                    )
                for k in range(8):
                    for j in range(4):
                        nc.tensor.matmul(
                            pzr_t[32 * j : 32 * j + BL, :],
                            hT[l][:, k, :],
                            w[:, k, bass.ts(j, 512)],
                            start=False,
                            stop=(k == 7),
                            tile_position=(0, 32 * j),
                            skip_group_check=True,
                        )
                zrG = stage.tile([128, 512], f32, tag="zrG")
                if biasT_sb[l] is not None:
                    szr = stage.tile([128, 512], f32, tag="szr")
                    nc.vector.tensor_add(szr[:], pzr_t[:], biasT_sb[l][0])
                    nc.scalar.activation(zrG[:], szr[:], AF.Sigmoid)
                else:
                    nc.scalar.activation(zrG[:], pzr_t[:], AF.Sigmoid)
                # transpose gates to H-on-partitions (PE transpose, fp32);
                # r half (source partitions 64:128) first so dh can start
                zrT = spsum.tile([128, 4, 128], f32, tag="zrT")
                for e in range(4):
                    nc.tensor.transpose(zrT[:, e, :], zrG[:, bass.ts(e, 128)], ident[:])

                # r*h -> lhsT for dh matmul (r strips: l in [64,128))
                rAP = zrT[:, :, 64:128].rearrange("p e (j b) -> p j e b", j=2)[
                    :, :, :, 0:BL
                ]
                rh = stage.tile([128, 8, BL], f32, tag="rh")
                rh4 = rh[:].rearrange("p (j e) b -> p j e b", j=2)
                hAP = hT[l][:].rearrange("p (j e) b -> p j e b", j=2)
                nc.vector.tensor_mul(rh4, rAP, hAP)

                # dh preactivation: ident-MM seed + 2 strip chains
                for j in range(2):
                    nc.tensor.matmul(
                        pdh_t[32 * j : 32 * j + BL, :],
                        ident8[:],
                        sxrow[:, bass.ds(2 * H + 512 * j, 512)],
                        start=True,
                        stop=False,
                        tile_position=(0, 32 * j),
                        skip_group_check=True,
                    )
                for k in range(8):
                    for j in range(2):
                        nc.tensor.matmul(
                            pdh_t[32 * j : 32 * j + BL, :],
                            rh[:, k, :],
                            w[:, k, bass.ds(2 * H + 512 * j, 512)],
                            start=False,
                            stop=(k == 7),
                            tile_position=(0, 32 * j),
                            skip_group_check=True,
                        )
                dhG = stage.tile([128, 512], f32, tag="dhG")
                if biasT_sb[l] is not None:
                    sdh = stage.tile([128, 512], f32, tag="sdh")
                    nc.vector.tensor_add(sdh[:], pdh_t[:], biasT_sb[l][1])
                    nc.scalar.activation(dhG[:], sdh[:], AF.Tanh)
                else:
                    nc.scalar.activation(dhG[:], pdh_t[:], AF.Tanh)
                dhT = spsum.tile([128, 4, 128], f32, tag="dhT")
                for e in range(4):
                    nc.tensor.transpose(dhT[:, e, :], dhG[:, bass.ts(e, 128)], ident[:])
                # copy dh out of PSUM so z (.) dh has a single PSUM operand
                dhS = stage.tile([128, 4, 128], f32, tag="dhS")
                nc.scalar.activation(dhS[:], dhT[:], AF.Copy)

                # h += eps * z (.) dh
                zAP = zrT[:, :, 0:64].rearrange("p e (j b) -> p j e b", j=2)[
                    :, :, :, 0:BL
                ]
                dAP = dhS[:, :, 0:64].rearrange("p e (j b) -> p j e b", j=2)[
                    :, :, :, 0:BL
                ]
                zdh = stage.tile([128, 8, BL], f32, tag="zdh")
                zdh4 = zdh[:].rearrange("p (j e) b -> p j e b", j=2)
                nc.vector.tensor_mul(zdh4, zAP, dAP)
                if eps[l] != 1.0:
                    nc.vector.tensor_scalar_mul(zdh[:], zdh[:], float(eps[l]))
                nc.vector.tensor_add(hT[l][:], hT[l][:], zdh[:])

                if l == 0:
                    nc.sync.dma_start(
                        y0T[:, :, bass.ds(t, 1), :].rearrange("hh p t b -> p hh t b"),
                        hT[l][:],
                    )
                else:
                    nc.sync.dma_start(
                        y1T_o[bass.ds(t, 1)].rearrange("t hh p b -> p t hh b"),
                        hT[l][:],
                    )

            with tc.For_i(0, NT, UNROLL, hint_engines=(mybir.EngineType.PE,)) as i0:
                for u in range(UNROLL):
                    emit_step(i0 + u, u % 2)

        # S0
        w_s0 = wslot()
        for k in range(8):
            nc.sync.dma_start(w_s0[:, k, 0 : 2 * H], uzr_p[0][k])
            nc.sync.dma_start(w_s0[:, k, 2 * H : 3 * H], aT_p[0][k])
        emit_scan(0, w_s0)

        # P1
        def p1_lhs(m):
            # rows [128m, 128m+128) = t in [16m, 16m+16)
            yt = xload.tile([128, 8, 128], f32, tag="yt")
            for k in range(8):
                nc.sync.dma_start(yt[:, k, :], y0T[k][:, bass.ts(m, 16), :])
            return yt

        emit_proj(1, 8, p1_lhs, proj[1])

        # S1
        w_s1 = wslot()
        for k in range(8):
            nc.sync.dma_start(w_s1[:, k, 0 : 2 * H], uzr_p[1][k])
            nc.sync.dma_start(w_s1[:, k, 2 * H : 3 * H], aT_p[1][k])
        emit_scan(1, w_s1)

        # ---- final hidden states ----
        for l in range(2):
            nc.sync.dma_start(hnT_o[l].rearrange("hh p b -> p hh b"), hT[l][:])

    nc.compile()
    return nc


def _prep_shared(inputs):
    f32 = np.float32
    shared = {"ident": np.eye(128, dtype=f32)}
    eps = []
    with_bias = []
    for l in range(2):
        s = str(l)
        Wz = np.asarray(inputs["Wz" + s], f32)
        Wr = np.asarray(inputs["Wr" + s], f32)
        Vh = np.asarray(inputs["Vh" + s], f32)
        Uz = np.asarray(inputs["Uz" + s], f32)
        Ur = np.asarray(inputs["Ur" + s], f32)
        Wh = np.asarray(inputs["Wh" + s], f32)
        A = Wh - Wh.T - GAMMA * np.eye(H, dtype=f32)
        Wall = np.concatenate([Wz, Wr, Vh], 0)  # (3H, K)
        K = Wall.shape[1]
        shared["wall" + s] = np.ascontiguousarray(Wall.T).reshape(K // 128, 128, 3 * H)
        shared["uzr" + s] = np.ascontiguousarray(
            np.concatenate([Uz, Ur], 0).T
        ).reshape(8, 128, 2 * H)
        shared["aT" + s] = np.ascontiguousarray(A.T).reshape(8, 128, H)
        b = np.concatenate(
            [np.asarray(inputs["bz" + s], f32), np.asarray(inputs["br" + s], f32),
             np.asarray(inputs["bh" + s], f32)]
        )
        wb = bool(np.any(b != 0.0))
        with_bias.append(wb)
        if wb:
            bt = np.zeros((2, 128, 512), f32)
            for j in range(4):
                bt[0, 32 * j : 32 * j + BL, :] = b[512 * j : 512 * (j + 1)][None, :]
            for j in range(2):
                bt[1, 32 * j : 32 * j + BL, :] = b[2048 + 512 * j : 2048 + 512 * (j + 1)][None, :]
            shared["biasT" + s] = bt
        eps.append(float(np.asarray(inputs["eps" + s], f32)))
    return shared, tuple(eps), tuple(with_bias)


def _core_inputs(inputs, shared, c, with_bias):
    f32 = np.float32
    x = np.asarray(inputs["x"], f32)[:, c * BL : (c + 1) * BL, :]  # (S, BL, I)
    S_ = x.shape[0]
    R = S_ * BL
    xT = np.ascontiguousarray(x.reshape(R, I).T).reshape(I // 128, 128, R)
    h0 = np.asarray(inputs["h0"], f32)[:, c * BL : (c + 1) * BL, :]  # (2, BL, H)
    h0T = np.ascontiguousarray(h0.transpose(0, 2, 1)).reshape(2, 8, 128, BL)
    m = {
        "xT": xT,
        "h0T": h0T,
        "ident": shared["ident"],
        "wall0": shared["wall0"],
        "wall1": shared["wall1"],
        "uzr0": shared["uzr0"],
        "uzr1": shared["uzr1"],
        "aT0": shared["aT0"],
        "aT1": shared["aT1"],
    }
    for l in range(2):
        if with_bias[l]:
            m[f"biasT{l}"] = shared[f"biasT{l}"]
    return m


def _assemble(results, S_=S):
    f32 = np.float32
    ys, hs = [], []
    for r in results:
        y1T = np.asarray(r["y1T"], f32)  # (S, 8, 128, BL)
        hnT = np.asarray(r["hnT"], f32)  # (2, 8, 128, BL)
        ys.append(np.transpose(y1T, (0, 3, 1, 2)).reshape(S_, BL, H))
        hs.append(np.transpose(hnT, (0, 3, 1, 2)).reshape(2, BL, H))
    y = np.concatenate(ys, axis=1)
    hn = np.concatenate(hs, axis=1)
    return y, hn


def _run(inputs, trace=False, tmpdir=None):
    from concourse.bass_utils import run_bass_kernel_spmd

    shared, eps, with_bias = _prep_shared(inputs)
    S_ = np.asarray(inputs["x"]).shape[0]
    nc = _build(S_, eps, with_bias)
    in_maps = [_core_inputs(inputs, shared, c, with_bias) for c in range(NCORES)]
    res = run_bass_kernel_spmd(
        nc,
        in_maps,
        core_ids=list(range(NCORES)),
        trace=trace,
        tmpdir=tmpdir,
    )
    y, hn = _assemble(res.results, S_)
    return (y, hn), res


def kernel(**inputs):
    (y, hn), _ = _run(inputs, trace=False)
    return y, hn


# revision 27
# speedup vs baseline: 1.0952x; 1.0952x over previous
"""Trainium2 Bass kernel for 2-layer AGRU (nn_AGRU_48584670052560).

The recurrence is strongly chaotic (a 1e-7 relative input perturbation
decorrelates the output by t=511), so the kernel must be faithful-fp32:
every matmul, activation and elementwise op runs in fp32; any bf16
shortcut saturates the error far above the fp32 envelope.

Structure (per core, data-parallel over batch, B=64 -> 8 x BL=8):
  P0: proj0 = x @ [Wz0;Wr0;Vh0].T          (batched rows, fp32)
  S0: 512-step scan, layer 0               (fp32)
  P1: proj1 = y0 @ [Wz1;Wr1;Vh1].T         (batched rows, fp32)
  S1: 512-step scan, layer 1 -> output

Scan-step dataflow: h.T is the stationary matmul operand (fp32
self-loading matmul); W.T slices stream. The four 512-wide N-chunks of
the z|r preactivation go to four PSUM partition strips (32j..32j+8) of
one PSUM bank via tile_position col packing so the streams overlap in
the PE array. Preactivations + projections are summed by DVE during
PSUM evacuation; gates come back to the H-on-partitions layout via
PE-mode transposes (fp32); all elementwise runs 128-partitions-wide.
Phases are sequential because both layers' fp32 U-matrices cannot be
SBUF-resident at once; one 96KB/partition weight slot is reused by all
four phases.
"""

import numpy as np

S, B, I, H, L = 512, 64, 512, 1024, 2
NCORES = 8
BL = B // NCORES  # 8
GAMMA = 0.01
UNROLL = 8  # steps per For_i body


def _build(S_=S, eps=(1.0, 1.0), with_bias=(False, False)):
    import concourse.mybir as mybir
    import concourse.tile as tile
    import concourse.bass as bass
    from concourse import bacc
    from contextlib import ExitStack

    f32 = mybir.dt.float32
    AF = mybir.ActivationFunctionType

    NT = S_
    R = S_ * BL  # rows = (t, b), t-major
    assert NT % UNROLL == 0

    nc = bacc.Bacc(None, target_bir_lowering=False)

    # ---- dram parameters (all fp32) ----
    xT_p = nc.declare_dram_parameter("xT", [I // 128, 128, R], f32, isOutput=False)
    wall_p = [
        nc.declare_dram_parameter("wall0", [I // 128, 128, 3 * H], f32, isOutput=False),
        nc.declare_dram_parameter("wall1", [H // 128, 128, 3 * H], f32, isOutput=False),
    ]
    uzr_p = [
        nc.declare_dram_parameter(f"uzr{l}", [8, 128, 2 * H], f32, isOutput=False)
        for l in range(2)
    ]
    aT_p = [
        nc.declare_dram_parameter(f"aT{l}", [8, 128, H], f32, isOutput=False)
        for l in range(2)
    ]
    h0T_p = nc.declare_dram_parameter("h0T", [2, 8, 128, BL], f32, isOutput=False)
    ident_p = nc.declare_dram_parameter("ident", [128, 128], f32, isOutput=False)
    biasT_p = []
    for l in range(2):
        if with_bias[l]:
            biasT_p.append(
                nc.declare_dram_parameter(f"biasT{l}", [2, 128, 512], f32, isOutput=False)
            )
        else:
            biasT_p.append(None)
    y1T_o = nc.declare_dram_parameter("y1T", [NT, 8, 128, BL], f32, isOutput=True)
    hnT_o = nc.declare_dram_parameter("hnT", [2, 8, 128, BL], f32, isOutput=True)

    with ExitStack() as ctx:
        tc = ctx.enter_context(tile.TileContext(nc))
        wpool = ctx.enter_context(tc.tile_pool(name="weights", bufs=1))
        state = ctx.enter_context(tc.tile_pool(name="state", bufs=1))
        dram = ctx.enter_context(tc.tile_pool(name="dram", bufs=1, space="DRAM"))
        spsum = ctx.enter_context(tc.tile_pool(name="spsum", bufs=1, space="PSUM"))
        ppsum = ctx.enter_context(tc.tile_pool(name="ppsum", bufs=2, space="PSUM"))
        stage = ctx.enter_context(tc.tile_pool(name="stage", bufs=2))
        xload = ctx.enter_context(tc.tile_pool(name="xload", bufs=3))

        # ---- DRAM scratch ----
        proj = [dram.tile([R, 3 * H], f32, tag=f"proj{l}", name=f"proj{l}") for l in range(2)]
        y0T = dram.tile([8, 128, NT, BL], f32, tag="y0T")

        # ---- persistent small tiles ----
        ident = state.tile([128, 128], f32, tag="ident", name="ident")
        nc.sync.dma_start(ident[:], ident_p[:])
        ident8 = state.tile([8, 8], f32, tag="ident8", name="ident8")
        nc.sync.dma_start(ident8[:], ident_p[0:8, 0:8])
        hT = [state.tile([128, 8, BL], f32, tag=f"hT{l}", name=f"hT{l}") for l in range(2)]
        for l in range(2):
            nc.sync.dma_start(hT[l][:], h0T_p[l].rearrange("hh p b -> p hh b"))
        biasT_sb = []
        for l in range(2):
            if with_bias[l]:
                bt = state.tile([2, 128, 512], f32, tag=f"biasT{l}", name=f"biasTs{l}")
                nc.sync.dma_start(bt[:], biasT_p[l][:])
                biasT_sb.append(bt)
            else:
                biasT_sb.append(None)

        # persistent scan psum/addend tiles (memset once; strips rewritten)
        pzr_t = spsum.tile([128, 512], f32, tag="pzr", name="pzr")
        pdh_t = spsum.tile([128, 512], f32, tag="pdh", name="pdh")
        nc.vector.memset(pzr_t[:], 0.0)
        nc.vector.memset(pdh_t[:], 0.0)
        sxzr_t = [state.tile([128, 512], f32, tag=f"sxzr{a}", name=f"sxzr{a}") for a in range(2)]
        sxdh_t = [state.tile([128, 512], f32, tag=f"sxdh{a}", name=f"sxdh{a}") for a in range(2)]
        for a in range(2):
            nc.vector.memset(sxzr_t[a][:], 0.0)
            nc.vector.memset(sxdh_t[a][:], 0.0)

        # ---- one 96KB/partition weight slot shared by all phases ----
        def wslot():
            return wpool.tile([128, 8, 3 * H], f32, tag="bigw", name="bigw")

        # ---- projection phase emitter ----
        def emit_proj(layer, wsrc_k, lhs_loader, out_dram):
            w = wslot()
            for k in range(wsrc_k):
                nc.sync.dma_start(w[:, k, :], wall_p[layer][k])
            nm = R // 128
            for m in range(nm):
                lt = lhs_loader(m)
                for n in range(6):
                    ps = ppsum.tile([128, 512], f32, tag="pj")
                    for k in range(wsrc_k):
                        nc.tensor.matmul(
                            ps[:],
                            lt[:, k, :],
                            w[:, k, bass.ts(n, 512)],
                            start=(k == 0),
                            stop=(k == wsrc_k - 1),
                        )
                    ob = stage.tile([128, 512], f32, tag="pjout", bufs=3)
                    if n % 2 == 0:
                        nc.scalar.activation(ob[:], ps[:], AF.Copy)
                    else:
                        nc.vector.tensor_copy(ob[:], ps[:])
                    nc.sync.dma_start(
                        out_dram[bass.ts(m, 128), bass.ts(n, 512)], ob[:]
                    )

        # P0
        def p0_lhs(m):
            xt = xload.tile([128, I // 128, 128], f32, tag="xt")
            for k in range(I // 128):
                nc.sync.dma_start(xt[:, k, :], xT_p[k][:, bass.ts(m, 128)])
            return xt

        emit_proj(0, I // 128, p0_lhs, proj[0])

        # ---- scan phase emitter ----
        def emit_scan(l, w):
            # w: weight slot with uzr in [:, :, 0:2048], aT in [:, :, 2048:3072]
            def emit_step(t, par):
                sxzr = sxzr_t[par]
                sxdh = sxdh_t[par]
                for j in range(4):
                    nc.sync.dma_start(
                        sxzr[32 * j : 32 * j + BL, :],
                        proj[l][bass.ts(t, BL), bass.ts(j, 512)],
                    )
                for j in range(2):
                    nc.sync.dma_start(
                        sxdh[32 * j : 32 * j + BL, :],
                        proj[l][bass.ts(t, BL), bass.ds(2048 + 512 * j, 512)],
                    )
                # z|r preactivation: 4 strip chains, k-interleaved
                for k in range(8):
                    for j in range(4):
                        nc.tensor.matmul(
                            pzr_t[32 * j : 32 * j + BL, :],
                            hT[l][:, k, :],
                            w[:, k, bass.ts(j, 512)],
                            start=(k == 0),
                            stop=(k == 7),
                            tile_position=(0, 32 * j),
                            skip_group_check=True,
                        )
                szr = stage.tile([128, 512], f32, tag="szr")
                nc.vector.tensor_add(szr[:], pzr_t[:], sxzr[:])
                if biasT_sb[l] is not None:
                    nc.vector.tensor_add(szr[:], szr[:], biasT_sb[l][0])
                zrG = stage.tile([128, 512], f32, tag="zrG")
                nc.scalar.activation(zrG[:], szr[:], AF.Sigmoid)
                # transpose gates to H-on-partitions (PE transpose, fp32);
                # r half (source partitions 64:128) first so dh can start
                zrT = spsum.tile([128, 4, 128], f32, tag="zrT")
                for e in range(4):
                    nc.tensor.transpose(zrT[:, e, :], zrG[:, bass.ts(e, 128)], ident[:])

                # r*h -> lhsT for dh matmul (r strips: l in [64,128))
                rAP = zrT[:, :, 64:128].rearrange("p e (j b) -> p j e b", j=2)[
                    :, :, :, 0:BL
                ]
                rh = stage.tile([128, 8, BL], f32, tag="rh")
                rh4 = rh[:].rearrange("p (j e) b -> p j e b", j=2)
                hAP = hT[l][:].rearrange("p (j e) b -> p j e b", j=2)
                nc.vector.tensor_mul(rh4, rAP, hAP)

                # dh preactivation: 2 strip chains
                for k in range(8):
                    for j in range(2):
                        nc.tensor.matmul(
                            pdh_t[32 * j : 32 * j + BL, :],
                            rh[:, k, :],
                            w[:, k, bass.ds(2 * H + 512 * j, 512)],
                            start=(k == 0),
                            stop=(k == 7),
                            tile_position=(0, 32 * j),
                            skip_group_check=True,
                        )
                sdh = stage.tile([128, 512], f32, tag="sdh")
                nc.vector.tensor_add(sdh[:], pdh_t[:], sxdh[:])
                if biasT_sb[l] is not None:
                    nc.vector.tensor_add(sdh[:], sdh[:], biasT_sb[l][1])
                dhG = stage.tile([128, 512], f32, tag="dhG")
                nc.scalar.activation(dhG[:], sdh[:], AF.Tanh)
                dhT = spsum.tile([128, 4, 128], f32, tag="dhT")
                for e in range(4):
                    nc.tensor.transpose(dhT[:, e, :], dhG[:, bass.ts(e, 128)], ident[:])
                # copy dh out of PSUM so z (.) dh has a single PSUM operand
                dhS = stage.tile([128, 4, 128], f32, tag="dhS")
                nc.scalar.activation(dhS[:], dhT[:], AF.Copy)

                # h += eps * z (.) dh
                zAP = zrT[:, :, 0:64].rearrange("p e (j b) -> p j e b", j=2)[
                    :, :, :, 0:BL
                ]
                dAP = dhS[:, :, 0:64].rearrange("p e (j b) -> p j e b", j=2)[
                    :, :, :, 0:BL
                ]
                zdh = stage.tile([128, 8, BL], f32, tag="zdh")
                zdh4 = zdh[:].rearrange("p (j e) b -> p j e b", j=2)
                nc.vector.tensor_mul(zdh4, zAP, dAP)
                if eps[l] != 1.0:
                    nc.vector.tensor_scalar_mul(zdh[:], zdh[:], float(eps[l]))
                nc.vector.tensor_add(hT[l][:], hT[l][:], zdh[:])

                if l == 0:
                    nc.sync.dma_start(
                        y0T[:, :, bass.ds(t, 1), :].rearrange("hh p t b -> p hh t b"),
                        hT[l][:],
                    )
                else:
                    nc.sync.dma_start(
                        y1T_o[bass.ds(t, 1)].rearrange("t hh p b -> p t hh b"),
                        hT[l][:],
                    )

            with tc.For_i(0, NT, UNROLL, hint_engines=(mybir.EngineType.PE,)) as i0:
                for u in range(UNROLL):
                    emit_step(i0 + u, u % 2)

        # S0
        w_s0 = wslot()
        for k in range(8):
            nc.sync.dma_start(w_s0[:, k, 0 : 2 * H], uzr_p[0][k])
            nc.sync.dma_start(w_s0[:, k, 2 * H : 3 * H], aT_p[0][k])
        emit_scan(0, w_s0)

        # P1
        def p1_lhs(m):
            # rows [128m, 128m+128) = t in [16m, 16m+16)
            yt = xload.tile([128, 8, 128], f32, tag="yt")
            for k in range(8):
                nc.sync.dma_start(yt[:, k, :], y0T[k][:, bass.ts(m, 16), :])
            return yt

        emit_proj(1, 8, p1_lhs, proj[1])

        # S1
        w_s1 = wslot()
        for k in range(8):
            nc.sync.dma_start(w_s1[:, k, 0 : 2 * H], uzr_p[1][k])
            nc.sync.dma_start(w_s1[:, k, 2 * H : 3 * H], aT_p[1][k])
        emit_scan(1, w_s1)

        # ---- final hidden states ----
        for l in range(2):
            nc.sync.dma_start(hnT_o[l].rearrange("hh p b -> p hh b"), hT[l][:])

    nc.compile()
    return nc


def _prep_shared(inputs):
    f32 = np.float32
    shared = {"ident": np.eye(128, dtype=f32)}
    eps = []
    with_bias = []
    for l in range(2):
        s = str(l)
        Wz = np.asarray(inputs["Wz" + s], f32)
        Wr = np.asarray(inputs["Wr" + s], f32)
        Vh = np.asarray(inputs["Vh" + s], f32)
        Uz = np.asarray(inputs["Uz" + s], f32)
        Ur = np.asarray(inputs["Ur" + s], f32)
        Wh = np.asarray(inputs["Wh" + s], f32)
        A = Wh - Wh.T - GAMMA * np.eye(H, dtype=f32)
        Wall = np.concatenate([Wz, Wr, Vh], 0)  # (3H, K)
        K = Wall.shape[1]
        shared["wall" + s] = np.ascontiguousarray(Wall.T).reshape(K // 128, 128, 3 * H)
        shared["uzr" + s] = np.ascontiguousarray(
            np.concatenate([Uz, Ur], 0).T
        ).reshape(8, 128, 2 * H)
        shared["aT" + s] = np.ascontiguousarray(A.T).reshape(8, 128, H)
        b = np.concatenate(
            [np.asarray(inputs["bz" + s], f32), np.asarray(inputs["br" + s], f32),
             np.asarray(inputs["bh" + s], f32)]
        )
        wb = bool(np.any(b != 0.0))
        with_bias.append(wb)
        if wb:
            bt = np.zeros((2, 128, 512), f32)
            for j in range(4):
                bt[0, 32 * j : 32 * j + BL, :] = b[512 * j : 512 * (j + 1)][None, :]
            for j in range(2):
                bt[1, 32 * j : 32 * j + BL, :] = b[2048 + 512 * j : 2048 + 512 * (j + 1)][None, :]
            shared["biasT" + s] = bt
        eps.append(float(np.asarray(inputs["eps" + s], f32)))
    return shared, tuple(eps), tuple(with_bias)


def _core_inputs(inputs, shared, c, with_bias):
    f32 = np.float32
    x = np.asarray(inputs["x"], f32)[:, c * BL : (c + 1) * BL, :]  # (S, BL, I)
    S_ = x.shape[0]
    R = S_ * BL
    xT = np.ascontiguousarray(x.reshape(R, I).T).reshape(I // 128, 128, R)
    h0 = np.asarray(inputs["h0"], f32)[:, c * BL : (c + 1) * BL, :]  # (2, BL, H)
    h0T = np.ascontiguousarray(h0.transpose(0, 2, 1)).reshape(2, 8, 128, BL)
    m = {
        "xT": xT,
        "h0T": h0T,
        "ident": shared["ident"],
        "wall0": shared["wall0"],
        "wall1": shared["wall1"],
        "uzr0": shared["uzr0"],
        "uzr1": shared["uzr1"],
        "aT0": shared["aT0"],
        "aT1": shared["aT1"],
    }
    for l in range(2):
        if with_bias[l]:
            m[f"biasT{l}"] = shared[f"biasT{l}"]
    return m


def _assemble(results, S_=S):
    f32 = np.float32
    ys, hs = [], []
    for r in results:
        y1T = np.asarray(r["y1T"], f32)  # (S, 8, 128, BL)
        hnT = np.asarray(r["hnT"], f32)  # (2, 8, 128, BL)
        ys.append(np.transpose(y1T, (0, 3, 1, 2)).reshape(S_, BL, H))
        hs.append(np.transpose(hnT, (0, 3, 1, 2)).reshape(2, BL, H))
    y = np.concatenate(ys, axis=1)
    hn = np.concatenate(hs, axis=1)
    return y, hn


def _run(inputs, trace=False, tmpdir=None):
    from concourse.bass_utils import run_bass_kernel_spmd

    shared, eps, with_bias = _prep_shared(inputs)
    S_ = np.asarray(inputs["x"]).shape[0]
    nc = _build(S_, eps, with_bias)
    in_maps = [_core_inputs(inputs, shared, c, with_bias) for c in range(NCORES)]
    res = run_bass_kernel_spmd(
        nc,
        in_maps,
        core_ids=list(range(NCORES)),
        trace=trace,
        tmpdir=tmpdir,
    )
    y, hn = _assemble(res.results, S_)
    return (y, hn), res


def kernel(**inputs):
    (y, hn), _ = _run(inputs, trace=False)
    return y, hn
